# revision 47
# baseline (speedup 1.0000x reference)
"""GCN embedding network kernel for Trainium2, sharded across 8 NeuronCores.

Math (derived from the reference GCN):
    A in {0,1}^{NxN};  deg[j] = colsum(A)[j] + 1;  dinv = 1/sqrt(deg)
    y  = x @ W1;  y'[i] = dinv[i] * y[i]
    z[j] = sum_i A[i,j] y'[i] + y'[j]              (Ahat^T aggregation + self loop)
    h[j] = relu(dinv[j] * z[j] + b1)
    u[i] = sum_j A[i,j] dinv[j] + dinv[i]
    r[i] = dinv[i] * u[i]                          (row sums of Ahat)
    out  = (sum_i r[i] h[i]) @ W2 + N * b2         -> [1, F_OUT]

Sharding: rows of A and x are sharded across 8 cores (2048 rows each).

Schedule (v4, ~730us/run measured via reps-NEFF pair slopes):
  Pass 1 streams the row block in 2-row-tile slabs (one batched DMA per
  slab); each jm-macro (2048 columns) is cast to an fp8 ring block
  (n_ring deep), column-summed on the PE with fp8 DoubleRow matmuls,
  and (for jm < NJ-n_ring) written to DRAM scratch on the scalar DMA
  queue.  A per-jm AllReduce of the column-sum partial is issued as
  soon as that jm finishes (all NJ blocks), so every core accumulates
  the full degree vector pipelined under the stream; the u accumulation
  (paired DVE mul + ACT accum-reduce against the fp8 ring block, bf16
  dinv) runs two jms behind the stream, interleaved per slab.  The last
  n_ring blocks stay resident in the ring and never touch DRAM.
  After the stream: scratch block 0 is prefetched into SBUF during the
  last AllReduce's latency window; each core selects its own block's
  degrees from the NJ AllReduce results with a partition-id mask
  (replaces the old ReduceScatter - no extra collective on the tail).
  Pass 2 computes z^T with 2-term-fp8-split DoubleRow matmuls; the hi
  and lo terms accumulate into the same 16 PSUM rows (PE-side fold),
  one ACT copy + one DVE add evacuate each jm, and a bf16
  ReduceScatter combines z^T.  The epilogue forms h (bf16), r and the
  [16] partial s_p; the host sums the 8 partials and applies the tiny
  W2/b2 head.

  Phase costs (pair-slope measured): A stream 445, +scr8 writes 63,
  +u 80, +post/epilogue 40, +z^T pass/RS 110 -> ~735us against a
  422us HBM roofline (134MB f32 read + 2x16.8MB fp8 scratch per core
  at 358GB/s).

  build_gcn(reps=K) unrolls the whole per-run body K times inside one
  NEFF (per-rep DRAM scratch, shared SBUF pools).  This exists purely
  for benchmarking: per-dispatch overhead through the axon proxy is
  ~1ms, comparable to one run, so timing differences K runs inside one
  NEFF is the only way to resolve the true per-run HW time.
"""

import numpy as np

import concourse.bass as bass
import concourse.bacc as bacc
import concourse.mybir as mybir
import concourse.tile as tile
from concourse.bass_utils import run_bass_kernel_spmd

# Problem constants (hardcoded per harness contract).
N = 16384
F_IN = 64
HID = 16
F_OUT = 32
NCORES = 8

FP = mybir.dt.float32
BF = mybir.dt.bfloat16
FP8 = mybir.dt.float8e4

AF = mybir.ActivationFunctionType
ALU = mybir.AluOpType
DR = mybir.MatmulPerfMode.DoubleRow


def build_gcn(n=N, ncores=NCORES, n_ring=4, skip_u=False, skip_zt=False,
              stop_after=None, u_slab=True, n_preload=2, reps=1,
              skip_coll=False, u_ttr=False, ulag=2, u_eng="mix2",
              scr_q="scalar", skip_scr=False, load_alt=False,
              cast_eng="dve"):
    """Build the SPMD Bass program. Returns the compiled Bacc object."""
    R = n // ncores            # rows per core == columns per j-block
    IT = R // 128              # 128-row i-tiles per core (16)
    NS = IT // 2               # 2-tile slabs per jm (8)
    JW = R                     # j-macro width
    NJ = ncores                # j-macros
    NB = NJ - n_ring           # blocks that bounce through DRAM scratch
    CW = 512                   # psum chunk width for matmuls
    NCH = JW // CW             # chunks per j-macro (4)
    groups = [list(range(ncores))]
    ULAG = ulag                # u runs this many jms behind the stream
    assert ULAG < n_ring, "u must finish before its ring block is recycled"
    A8B = 4 if n_ring <= 3 else 2   # prefetch half-block buffers
    _lvl = {None: 99, "p1": 1, "p2": 2}[stop_after]
    n_preload = min(n_preload, A8B, 2 * NB)

    nc = bacc.Bacc("TRN2", target_bir_lowering=False, debug=False,
                   num_devices=ncores)

    a_t = nc.dram_tensor("A_blk", [R, n], FP, kind="ExternalInput")
    x_t = nc.dram_tensor("x_blk", [R, F_IN], FP, kind="ExternalInput")
    w1_t = nc.dram_tensor("W1", [F_IN, HID], FP, kind="ExternalInput")
    b1_t = nc.dram_tensor("b1", [HID], FP, kind="ExternalInput")
    s_t = nc.dram_tensor("s_out", [HID, 1], FP, kind="ExternalOutput")

    ident_t = nc.inline_tensor(np.eye(128, dtype=np.float32), name="ident")
    iota_t = nc.inline_tensor(
        np.tile(np.arange(ncores, dtype=np.float32), (128, 1)), name="iota8")

    with tile.TileContext(nc) as tc:
        with tc.tile_pool(name="glob", bufs=1) as g, \
             tc.tile_pool(name="ring", bufs=n_ring) as ringp, \
             tc.tile_pool(name="dram", bufs=1, space="DRAM") as dram:
            # ---- persistent tiles (shared across reps) ----
            ident_sb = g.tile([128, 128], FP)
            ident_bf = g.tile([HID, HID], BF)
            ones8 = g.tile([128, 2, 16], FP8)
            w1_sb = g.tile([F_IN, HID], FP)
            b1_sb = g.tile([HID, 1], FP)
            y3 = g.tile([128, IT, HID], FP)
            ypT_sb = g.tile([HID, R], BF)
            yp8 = g.tile([128, IT, 64], FP8)
            tmpa = g.tile([128, IT, HID], BF)
            red_all = g.tile([128, IT * NJ], FP)
            u_sb = g.tile([128, IT], FP)
            dinv_blk = g.tile([128, IT], FP)

            r_sb = g.tile([128, IT], BF)
            s_sb = g.tile([HID, 1], FP)
            pid_u = g.tile([128, 1], mybir.dt.uint32)
            mask_sb = g.tile([128, NJ], FP)
            csjs = g.tile([128, IT, NJ], FP)

            nc.sync.dma_start(ident_sb[:, :], ident_t.ap())
            nc.vector.tensor_copy(ident_bf[:, :],
                                  ident_sb[0:HID, 0:HID])
            nc.vector.memset(ones8[:, :, :], 1.0)
            nc.vector.memset(yp8[:, :, :], 0.0)
            nc.sync.dma_start(w1_sb[:, :], w1_t.ap())
            nc.sync.dma_start(b1_sb[:, :],
                              b1_t.ap().rearrange("(p f) -> p f", f=1))
            # mask_sb[p, j] = 1.0 iff j == partition(core) id
            nc.sync.dma_start(
                pid_u[:, :],
                bass.AP(nc.partition_id_tensor, 0, [[0, 128], [1, 1]]))
            nc.sync.dma_start(mask_sb[:, :], iota_t.ap())
            pid_f = g.tile([128, 1], FP)
            nc.vector.tensor_copy(pid_f[:, :], pid_u[:, :])
            nc.vector.tensor_scalar(mask_sb[:, :], mask_sb[:, :],
                                    pid_f[:, 0:1], None, ALU.is_equal)

            for rp in range(reps):
                one_rep(nc, tc, locals())

    nc.compile()
    return nc


def one_rep(nc, tc, env):
    """One full per-run body (pass 1 + collectives + pass 2 + epilogue).

    `env` is build_gcn's locals(); tile/pool names are suffixed with the
    rep index so reps>1 unrolls cleanly inside one NEFF.
    """
    (n, ncores, n_ring, skip_u, skip_zt, u_slab, n_preload, rp,
     skip_coll, u_ttr, u_eng, scr_q, skip_scr, load_alt, cast_eng) = (
        env[k] for k in ("n", "ncores", "n_ring", "skip_u", "skip_zt",
                         "u_slab", "n_preload", "rp", "skip_coll", "u_ttr",
                         "u_eng", "scr_q", "skip_scr", "load_alt",
                         "cast_eng"))
    (R, IT, NS, JW, NJ, NB, CW, NCH, groups, ULAG, A8B, _lvl) = (
        env[k] for k in ("R", "IT", "NS", "JW", "NJ", "NB", "CW", "NCH",
                         "groups", "ULAG", "A8B", "_lvl"))
    (a_t, x_t, s_t, dram, ringp) = (
        env[k] for k in ("a_t", "x_t", "s_t", "dram", "ringp"))
    (ident_sb, ident_bf, ones8, w1_sb, b1_sb, y3, ypT_sb, yp8, tmpa,
     red_all, u_sb, dinv_blk, r_sb, s_sb, mask_sb, csjs) = (
        env[k] for k in ("ident_sb", "ident_bf", "ones8", "w1_sb", "b1_sb",
                         "y3", "ypT_sb", "yp8", "tmpa", "red_all", "u_sb",
                         "dinv_blk", "r_sb", "s_sb", "mask_sb", "csjs"))

    def nm(s):
        return f"{s}_r{rp}"

    # ---- per-rep DRAM scratch (collective buffers must be distinct) ----
    djnv = dram.tile([JW], FP, name=nm("djnv"))
    csj_in = [dram.tile([JW], FP, name=nm(f"csj_in_{j}"))
              for j in range(NJ)]
    csj_ar = [dram.tile([JW], FP, addr_space="Shared",
                        name=nm(f"csj_ar_{j}")) for j in range(NJ)]
    if NB > 0:
        scr8 = dram.tile([NB, 128, IT, JW], FP8, name=nm("scr8"))
    zt_in = dram.tile([ncores, HID, JW], BF, name=nm("zt_in"))
    zt_rs = dram.tile([HID, JW], BF, name=nm("zt_rs"))

    # ---- y = x @ W1 (unscaled; dinv applied post-RS) ----
    with tc.tile_pool(name=nm("yb"), bufs=2) as yb, \
         tc.tile_pool(name=nm("ybps"), bufs=2, space="PSUM") as ybps:
        x3 = yb.tile([128, IT, F_IN], FP, name=nm("x3"))
        nc.sync.dma_start(
            x3[:, :, :],
            x_t.ap().rearrange("(it p) c -> p it c", p=128))
        for it in range(IT):
            xt_ps = ybps.tile([F_IN, 128], FP, name=nm(f"xt_ps_{it}"),
                              tag="xt_ps")
            nc.tensor.transpose(xt_ps[:, :], x3[:, it, :],
                                ident_sb[:, :])
            xt_sb = yb.tile([F_IN, 128], FP, name=nm(f"xt_sb_{it}"),
                            tag="xt_sb")
            nc.vector.tensor_copy(xt_sb[:, :], xt_ps[:, :])
            y_ps = ybps.tile([128, HID], FP, name=nm(f"y_ps_{it}"),
                             tag="y_ps")
            nc.tensor.matmul(y_ps[:, :], xt_sb[:, :], w1_sb[:, :],
                             start=True, stop=True)
            nc.vector.tensor_copy(y3[:, it, :], y_ps[:, :])

    # ============ pass 1: stream A, colsum, cast, u ============
    ring = []
    dsegs = {}

    def u_head(jm, up):
        """dinv segment for block jm (bf16, broadcast)."""
        dseg = up.tile([128, JW], BF, name=nm(f"dseg_{jm}"),
                       tag="dseg", bufs=2)
        dsrc_t = (csj_in[jm] if skip_coll else csj_ar[jm]).tensor
        dsrc = bass.AP(dsrc_t, 0, [[0, 128], [1, JW]])
        nc.gpsimd.dma_start(dseg[:, :], dsrc)
        with nc.allow_low_precision(reason="bf16 dinv for u is "
                                    "well within the 2e-2 gate"):
            nc.scalar.activation(dseg[:, :], dseg[:, :], AF.Sqrt,
                                 bias=1.0)
            nc.vector.reciprocal(dseg[:, :], dseg[:, :])
        dsegs[jm] = dseg

    red3 = red_all[:, :].rearrange("p (it j) -> p it j", j=NJ)

    def u_tiles(jm, its, up):
        """u partial rows for block jm, i-tiles `its`."""
        dseg = dsegs[jm]
        if u_eng in ("gp2", "dve2", "mix2"):
            # one [128, 2, JW] mul per slab pair
            assert len(its) == 2 and its[1] == its[0] + 1
            it0 = its[0]
            prod = up.tile([128, 2, JW], BF, name=nm(f"prod_{jm}_{it0}"),
                           tag="prod", bufs=2 if u_eng != "dve2" else 1)
            dbc = dseg[:, :].unsqueeze(1).broadcast_to([128, 2, JW])
            eng = nc.gpsimd if u_eng == "gp2" else nc.vector
            eng.tensor_mul(prod[:, :, :], ring[jm][:, it0:it0 + 2, :], dbc)
            if u_eng == "mix2":
                # reduce on ACT (accum copy), one per i-tile
                for t in range(2):
                    k = (it0 + t) * NJ + jm
                    nc.scalar.activation(prod[:, t, :], prod[:, t, :],
                                         AF.Copy,
                                         accum_out=red_all[:, k:k + 1])
            else:
                nc.vector.tensor_reduce(red3[:, it0:it0 + 2, jm],
                                        prod[:, :, :],
                                        mybir.AxisListType.X, ALU.add)
            return
        for it in its:
            prod = up.tile([128, JW], BF, name=nm(f"prod_{jm}_{it}"),
                           tag="prod", bufs=2)
            k = it * NJ + jm
            if u_ttr:
                nc.vector.tensor_tensor_reduce(
                    prod[:, :], ring[jm][:, it, :], dseg[:, :], 1.0, 0.0,
                    ALU.mult, ALU.add, red_all[:, k:k + 1])
            else:
                nc.vector.tensor_mul(prod[:, :], ring[jm][:, it, :],
                                     dseg[:, :])
                nc.scalar.activation(prod[:, :], prod[:, :], AF.Copy,
                                     accum_out=red_all[:, k:k + 1])

    def u_block(jm, up):
        u_head(jm, up)
        if u_eng in ("gp2", "dve2", "mix2"):
            for it0 in range(0, IT, 2):
                u_tiles(jm, (it0, it0 + 1), up)
        else:
            u_tiles(jm, range(IT), up)

    def csj_fetch(jm):
        """own-block deg candidates: csjs[:, :, jm] <- AR result for jm."""
        src = csj_in[jm] if skip_coll else csj_ar[jm]
        nc.gpsimd.dma_start(
            csjs[:, :, jm],
            bass.AP(src.tensor, 0, [[1, 128], [128, IT]]))

    preloaded = {}
    p2pool_cm = tc.tile_pool(name=nm("p2"), bufs=2)
    p2pool = None
    with tc.tile_pool(name=nm("up"), bufs=1) as up:
        with tc.tile_pool(name=nm("p1"), bufs=2) as p1, \
             tc.tile_pool(name=nm("p1ps"), bufs=2, space="PSUM") as p1ps:
            for jm in range(NJ):
                a8 = ringp.tile([128, IT, JW], FP8, name=nm(f"a8_{jm}"),
                                tag="a8ring")
                ring.append(a8)
                cs_ps = p1ps.tile([16, JW], FP, name=nm(f"cs_ps_{jm}"),
                                  tag="cs_ps")
                for s in range(NS):
                    slab = p1.tile([128, 2, JW], FP,
                                   name=nm(f"a_{jm}_{s}"), tag="aslab")
                    ld_eng = (nc.scalar if (load_alt and s % 2) else nc.sync)
                    ld_eng.dma_start(
                        slab[:, :, :],
                        a_t.ap()[s * 256:(s + 1) * 256,
                                 jm * JW:(jm + 1) * JW]
                        .rearrange("(t p) c -> p t c", p=128))
                    cst = nc.gpsimd if cast_eng == "gp" else nc.vector
                    cst.tensor_copy(a8[:, 2 * s:2 * s + 2, :],
                                    slab[:, :, :])
                    for c in range(NCH):
                        nc.tensor.matmul(
                            cs_ps[:, c * CW:(c + 1) * CW],
                            ones8[:, :, :],
                            a8[:, 2 * s:2 * s + 2,
                               c * CW:(c + 1) * CW],
                            start=(s == 0), stop=(s == NS - 1),
                            perf_mode=DR)
                    if jm < NB and not skip_scr:
                        scr_eng = {"sync": nc.sync, "gp": nc.gpsimd,
                                   "scalar": nc.scalar}[scr_q]
                        scr_eng.dma_start(
                            scr8[jm, :, 2 * s:2 * s + 2, :],
                            a8[:, 2 * s:2 * s + 2, :])
                    if (not skip_u and jm >= ULAG and u_slab):
                        if s == 0:
                            u_head(jm - ULAG, up)
                        u_tiles(jm - ULAG, (2 * s, 2 * s + 1), up)
                cs_tmp = p1.tile([1, JW], FP, name=nm(f"cs_tmp_{jm}"),
                                 tag="cs_tmp", bufs=1)
                nc.scalar.activation(cs_tmp[0:1, :], cs_ps[0:1, :],
                                     AF.Copy)
                nc.scalar.dma_start(csj_in[jm].unsqueeze(0),
                                    cs_tmp[0:1, :])
                if jm >= ULAG:
                    csj_fetch(jm - ULAG)
                if not skip_coll:
                    nc.gpsimd.collective_compute(
                        "AllReduce", ALU.add, replica_groups=groups,
                        ins=[csj_in[jm].opt()],
                        outs=[csj_ar[jm].opt()])
                if not skip_u and jm >= ULAG and not u_slab:
                    u_block(jm - ULAG, up)

        if not skip_u:
            for uj in range(NJ - ULAG, NJ - 1):
                u_block(uj, up)
        p2pool = p2pool_cm.__enter__()
        if _lvl >= 2 and not skip_zt:
            # prefetch scratch halves into the RS latency window
            for k in range(n_preload):
                pjm, phf = divmod(k, 2)
                sh = p2pool.tile([128, IT // 2, JW], FP8,
                                 name=nm(f"a8s_{pjm}_{phf}"),
                                 tag="a8s", bufs=A8B)
                nc.gpsimd.dma_start(
                    sh[:, :, :],
                    scr8[pjm, :, phf * (IT // 2):
                         (phf + 1) * (IT // 2), :])
                preloaded[(pjm, phf)] = sh
        for j in range(NJ - ULAG, NJ):
            csj_fetch(j)

        # ---- post-AR: pid-masked select of own deg, dinv, y' split ----
        if _lvl >= 2:
            nc.vector.tensor_scalar(dinv_blk[:, :], csjs[:, :, 0],
                                    mask_sb[:, 0:1], None, ALU.mult)
            for j in range(1, NJ):
                nc.vector.scalar_tensor_tensor(
                    dinv_blk[:, :], csjs[:, :, j], mask_sb[:, j:j + 1],
                    dinv_blk[:, :], ALU.mult, ALU.add)
            nc.scalar.activation(dinv_blk[:, :], dinv_blk[:, :],
                                 AF.Sqrt, bias=1.0)
            nc.vector.reciprocal(dinv_blk[:, :], dinv_blk[:, :])
            # dinv broadcast over partitions for the epilogue: DRAM bounce
            nc.scalar.dma_start(
                bass.AP(djnv.tensor, 0, [[1, 128], [128, IT]]),
                dinv_blk[:, :])
            with tc.tile_pool(name=nm("ppps"), bufs=1,
                              space="PSUM") as ppps:
                dbc = dinv_blk[:, :].unsqueeze(2).broadcast_to(
                    [128, IT, HID])
                nc.vector.tensor_mul(y3[:, :, :], y3[:, :, :], dbc)
                nc.vector.tensor_copy(yp8[:, :, 0:HID], y3[:, :, :])
                nc.vector.tensor_sub(tmpa[:, :, :], y3[:, :, :],
                                     yp8[:, :, 0:HID])
                nc.vector.tensor_copy(yp8[:, :, 32:32 + HID],
                                      tmpa[:, :, :])
                ypt3 = ppps.tile([HID, IT, 128], FP, name=nm("ypt3"))
                for it in range(IT):
                    nc.tensor.transpose(ypt3[:, it, :], y3[:, it, :],
                                        ident_sb[:, :])
                nc.vector.tensor_copy(
                    ypT_sb[:, :].rearrange("f (it p) -> f it p",
                                           p=128),
                    ypt3[:, :, :])
        # ============ pass 2: z^T aggregation ============
        if _lvl >= 2 and not skip_zt:
            assert p2pool is not None
            bounce, rings = list(range(NB)), list(range(NB, NJ))
            zt_order = []
            while bounce or rings:
                if bounce:
                    zt_order.append(bounce.pop(0))
                if rings:
                    zt_order.append(rings.pop(0))
            HS = NS // 2
            p2 = p2pool
            with tc.tile_pool(name=nm("p2ps"), bufs=2,
                              space="PSUM") as p2ps:
                if not skip_u:
                    u_block(NJ - 1, up)
                for jm in zt_order:
                    if jm < NB:
                        halves = []
                        for hf in range(2):
                            if (jm, hf) in preloaded:
                                halves.append(preloaded[(jm, hf)])
                                continue
                            sh = p2.tile([128, IT // 2, JW], FP8,
                                         name=nm(f"a8s_{jm}_{hf}"),
                                         tag="a8s", bufs=A8B)
                            nc.sync.dma_start(
                                sh[:, :, :],
                                scr8[jm, :, hf * (IT // 2):
                                     (hf + 1) * (IT // 2), :])
                            halves.append(sh)

                        def msrc(p, c, hv=halves):
                            hf, pl = divmod(p, HS)
                            return hv[hf][:, 2 * pl:2 * pl + 2,
                                          c * CW:(c + 1) * CW]
                    else:
                        def msrc(p, c, rs=ring[jm]):
                            return rs[:, 2 * p:2 * p + 2,
                                      c * CW:(c + 1) * CW]
                    zt_ps = p2ps.tile([64, JW], FP,
                                      name=nm(f"zt_ps_{jm}"),
                                      tag="zt_ps")
                    for p in range(NS):
                        for c in range(NCH):
                            nc.tensor.matmul(
                                zt_ps[:, c * CW:(c + 1) * CW],
                                yp8[:, 2 * p:2 * p + 2, :],
                                msrc(p, c),
                                start=(p == 0),
                                stop=(p == NS - 1),
                                perf_mode=DR)
                    zt_sb = p2.tile([HID, JW], BF,
                                    name=nm(f"zt_sb_{jm}"), tag="zt_sb",
                                    bufs=2)
                    nc.scalar.activation(zt_sb[:, :], zt_ps[0:HID, :],
                                         AF.Copy)
                    nc.vector.tensor_add(zt_sb[:, :], zt_sb[:, :],
                                         zt_ps[32:32 + HID, :])
                    nc.gpsimd.dma_start(zt_in[jm, :, :],
                                        zt_sb[:, :])
            nc.gpsimd.collective_compute(
                "ReduceScatter", ALU.add, replica_groups=groups,
                ins=[zt_in.opt()], outs=[zt_rs.opt()])
        p2pool_cm.__exit__(None, None, None)

    # ============ epilogue: h, r, s_p ============
    if _lvl >= 99:
        with tc.tile_pool(name=nm("fin"), bufs=2) as fin, \
             tc.tile_pool(name=nm("fps"), bufs=2, space="PSUM") as fps, \
             tc.tile_pool(name=nm("sps"), bufs=1, space="PSUM") as sps:
            hT_sb = ypT_sb
            down_f = fin.tile([HID, JW], FP, name=nm("down_f"), bufs=1)
            nc.gpsimd.dma_start(
                down_f[:, :],
                bass.AP(djnv.tensor, 0, [[0, HID], [1, JW]]))
            if not skip_zt:
                zt_rb = fin.tile([HID, JW], BF, name=nm("zt_rb"))
                nc.sync.dma_start(zt_rb[:, :], zt_rs[:, :])
                nc.vector.tensor_add(hT_sb[:, :], zt_rb[:, :],
                                     ypT_sb[:, :])
            nc.vector.tensor_mul(hT_sb[:, :], hT_sb[:, :],
                                 down_f[:, :])
            nc.scalar.activation(hT_sb[:, :], hT_sb[:, :], AF.Relu,
                                 bias=b1_sb[:, 0:1])
            # u = sum_jm red_all; r = dinv * (u + dinv)
            if skip_u:
                nc.vector.memset(u_sb[:, :], 0.0)
            else:
                ra = red_all[:, :].rearrange(
                    "p (it j) -> p it j", j=NJ)
                nc.vector.tensor_copy(u_sb[:, :], ra[:, :, 0])
                for j in range(1, NJ):
                    nc.vector.tensor_add(u_sb[:, :], u_sb[:, :],
                                         ra[:, :, j])
            with nc.allow_low_precision(reason="bf16 r; "
                                        "error ~2^-9 << gate"):
                nc.vector.tensor_add(r_sb[:, :], u_sb[:, :],
                                     dinv_blk[:, :])
                nc.vector.tensor_mul(r_sb[:, :], r_sb[:, :],
                                     dinv_blk[:, :])
            s_ps = sps.tile([HID, 1], FP, name=nm("s_ps"))
            h3_ps = fps.tile([128, IT, HID], BF, name=nm("h3_ps"))
            for it in range(IT):
                nc.tensor.transpose(
                    h3_ps[:, it, :],
                    hT_sb[:, it * 128:(it + 1) * 128],
                    ident_bf[:, :])
            h3_sb = fin.tile([128, IT, HID], BF, name=nm("h3_sb"))
            nc.vector.tensor_copy(h3_sb[:, :, :], h3_ps[:, :, :])
            for it in range(IT):
                nc.tensor.matmul(s_ps[:, :], h3_sb[:, it, :],
                                 r_sb[:, it:it + 1],
                                 start=(it == 0), stop=(it == IT - 1))
            nc.vector.tensor_copy(s_sb[:, :], s_ps[:, :])
            nc.sync.dma_start(s_t.ap(), s_sb[:, :])


_NC_CACHE = {}


def _get_nc(**kw):
    key = tuple(sorted(kw.items()))
    if key not in _NC_CACHE:
        _NC_CACHE[key] = build_gcn(**kw)
    return _NC_CACHE[key]


def kernel(A, x, W1, b1, W2, b2, _trace=False, **build_kw):
    """Full-input entry point: shards internally across 8 NeuronCores."""
    n = A.shape[0]
    R = n // NCORES
    nc = _get_nc(n=n, **build_kw)

    in_maps = []
    for c in range(NCORES):
        in_maps.append({
            "A_blk": np.ascontiguousarray(A[c * R:(c + 1) * R], np.float32),
            "x_blk": np.ascontiguousarray(x[c * R:(c + 1) * R], np.float32),
            "W1": np.ascontiguousarray(W1, np.float32),
            "b1": np.ascontiguousarray(b1, np.float32),
        })
    res = run_bass_kernel_spmd(nc, in_maps, core_ids=list(range(NCORES)),
                               trace=_trace)
    s = np.zeros(HID, np.float32)
    for c in range(NCORES):
        s = s + res.results[c]["s_out"].ravel().astype(np.float32)
    out = s @ np.asarray(W2, np.float32) + np.float32(n) * np.asarray(
        b2, np.float32)
    if _trace:
        kernel.last_results = res
    return out[None, :].astype(np.float32)
